# revision 1
# baseline (speedup 1.0000x reference)
"""Depthwise 3x3 conv + sync BatchNorm (train mode) + ReLU6 on 8 Trainium2 cores.

Sharding: channels (192) split 24-per-core. Depthwise conv and BN are
per-channel independent, so no cross-core communication is needed.

Per-channel device pipeline (each core, 24 channels):
  - DMA in: zero-padded x slab [114, 32, 114] (H in partitions).
  - Conv as banded matmuls: for each W-tap dj, lhsT A_dj[k, m] = w[k-m, dj]
    (3-diagonal band). 8 image-groups of 4 accumulate 3 taps each in PSUM
    ([112, 448] = one bank per group).
  - ScalarE drains PSUM->SBUF with fused accum_out (per-partition sum).
  - DVE tensor_tensor_reduce computes per-partition sum of y^2.
  - Partition-collapse via ones-matmul -> scalar mean/var chain -> per-channel
    scale' = gamma*rsqrt(var+eps), bias' = beta - mean*scale' (conv bias b
    cancels exactly in train-mode BN, so it is never applied).
  - Broadcast (outer-product matmul) scale'/bias' to [112,1]; ScalarE applies
    Relu(scale'*y + bias'); gpsimd clamps to 6.0; DMA out.
"""

import numpy as np
from contextlib import ExitStack

import concourse.bass as bass
import concourse.mybir as mybir
import concourse.tile as tile
from concourse import bacc, bass_utils

FP32 = mybir.dt.float32
FP32R = mybir.dt.float32r  # tf32-style: full-rate PE streaming (fp32 is 1/4)
AF = mybir.ActivationFunctionType
ALU = mybir.AluOpType

N, C, H, W = 32, 192, 112, 112
NCORES = 8
CPC = C // NCORES          # 24 channels per core
HP, WP = H + 2, W + 2      # zero-padded spatial dims
G = 8                      # image groups (PSUM banks) per channel
IPG = N // G               # 4 images per group
NF = IPG * W               # 448 matmul free dim (fp32 PSUM bank limit 512)
NTOT = N * H * W           # BN reduction size per channel
BN_EPS = 1e-5


def _emit(ctx: ExitStack, tc, nc, x_d, a_d, gb_d, o_d, n_ch):
    a_pool = ctx.enter_context(tc.tile_pool(name="a", bufs=1))
    const_pool = ctx.enter_context(tc.tile_pool(name="const", bufs=1))
    x_pool = ctx.enter_context(tc.tile_pool(name="x", bufs=3))
    y_pool = ctx.enter_context(tc.tile_pool(name="y", bufs=4))
    z_pool = ctx.enter_context(tc.tile_pool(name="z", bufs=2))
    st_pool = ctx.enter_context(tc.tile_pool(name="st", bufs=3))
    sc_pool = ctx.enter_context(tc.tile_pool(name="sc", bufs=3))
    st = {c: {} for c in range(n_ch)}
    psum_y = ctx.enter_context(tc.tile_pool(name="py", bufs=5, space="PSUM"))
    psum_s = ctx.enter_context(tc.tile_pool(name="ps", bufs=2, space="PSUM"))
    psum_b = ctx.enter_context(tc.tile_pool(name="pb", bufs=1, space="PSUM"))

    a_all = a_pool.tile([HP, n_ch, 3, W], FP32R)
    nc.sync.dma_start(a_all[:], a_d.ap())
    gb = const_pool.tile([1, 2 * n_ch], FP32)
    nc.sync.dma_start(gb[:], gb_d.ap())
    ones_col = const_pool.tile([H, 1], FP32)   # lhsT for partition collapse
    nc.vector.memset(ones_col[:], 1.0)
    ones_row = const_pool.tile([1, H], FP32)   # lhsT for partition broadcast
    nc.vector.memset(ones_row[:], 1.0)
    eps_t = const_pool.tile([1, 1], FP32)      # BN eps as Sqrt bias operand
    nc.vector.memset(eps_t[:], BN_EPS)

    def emit_conv(c):
        x_t = x_pool.tile([HP, N, WP], FP32R)
        nc.sync.dma_start(x_t[:], x_d.ap()[c])
        y_sb = y_pool.tile([H, G, NF], FP32)
        bst = st_pool.tile([H, G, 6], FP32, tag="bst")
        for g in range(G):
            pt = psum_y.tile([H, NF], FP32, tag="pt")
            for dj in range(3):
                nc.tensor.matmul(
                    pt[:],
                    a_all[:, c, dj, :],
                    x_t[:, g * IPG:(g + 1) * IPG, dj:dj + W],
                    start=(dj == 0),
                    stop=(dj == 2),
                )
            nc.scalar.activation(y_sb[:, g, :], pt[:], AF.Copy, bias=0.0)
            nc.vector.bn_stats(bst[:, g, :], y_sb[:, g, :])
        stats3 = st_pool.tile([H, 3], FP32, tag="stats3")
        nc.vector.bn_aggr(stats3[:, 0:2], bst[:])
        nc.vector.tensor_scalar(
            stats3[:, 2:3], stats3[:, 0:1], stats3[:, 0:1], None, op0=ALU.mult
        )
        st[c].update(y=y_sb, stats3=stats3)

    def emit_fin1(c):
        # partition collapse + per-channel scalar chain -> scpair
        stats3 = st[c]["stats3"]
        pst = psum_s.tile([1, 3], FP32, tag="pst")
        nc.tensor.matmul(pst[:], ones_col[:], stats3[:])
        em = sc_pool.tile([1, 3], FP32, tag="em")
        nc.vector.tensor_scalar_mul(em[:], pst[:], 1.0 / H)
        m2 = sc_pool.tile([1, 1], FP32, tag="m2")
        nc.vector.tensor_scalar(m2[:], em[:, 0:1], em[:, 0:1], None, op0=ALU.mult)
        varr = sc_pool.tile([1, 1], FP32, tag="varr")
        nc.vector.tensor_scalar(
            varr[:], em[:, 1:2], em[:, 2:3], m2[:], op0=ALU.add, op1=ALU.subtract
        )
        std = sc_pool.tile([1, 1], FP32, tag="std")
        nc.scalar.activation(std[:], varr[:], AF.Sqrt, bias=eps_t[:])
        istd = sc_pool.tile([1, 1], FP32, tag="istd")
        nc.vector.reciprocal(istd[:], std[:])
        scpair = sc_pool.tile([1, 2], FP32, tag="scpair")
        nc.vector.tensor_scalar(
            scpair[:, 0:1], istd[:], gb[:, c:c + 1], None, op0=ALU.mult
        )
        msc = sc_pool.tile([1, 1], FP32, tag="msc")
        nc.vector.tensor_scalar(
            msc[:], em[:, 0:1], scpair[:, 0:1], None, op0=ALU.mult
        )
        nc.vector.tensor_scalar(
            scpair[:, 1:2], gb[:, n_ch + c:n_ch + c + 1], msc[:], None,
            op0=ALU.subtract,
        )
        st[c]["scpair"] = scpair

    def emit_fin2(c):
        # broadcast scale'/bias' across partitions (outer product)
        pb = psum_b.tile([H, 2], FP32, tag="pb")
        nc.tensor.matmul(pb[:], ones_row[:], st[c]["scpair"][:])
        bc = sc_pool.tile([H, 2], FP32, tag="bc")
        nc.vector.tensor_copy(bc[:], pb[:])
        st[c]["bc"] = bc

    def emit_out(c):
        y_sb, bc = st[c]["y"], st[c]["bc"]
        z_sb = z_pool.tile([H, G, IPG, W], FP32, tag="z")
        hg = G // 2
        for h2 in range(2):
            zf = z_sb[:, h2 * hg:(h2 + 1) * hg].rearrange("p g i w -> p (g i w)")
            nc.scalar.activation(
                zf,
                y_sb[:, h2 * hg:(h2 + 1) * hg, :].rearrange("p g f -> p (g f)"),
                AF.Relu, bias=bc[:, 1:2], scale=bc[:, 0:1],
            )
            nc.vector.tensor_scalar_min(zf, zf, 6.0)
            # SWDGE ring: keeps the in-order Sync ring free for x prefetches
            nc.gpsimd.dma_start(
                o_d.ap()[c].rearrange("h (s n) w -> h s n w", s=2)[:, h2],
                z_sb[:, h2 * hg:(h2 + 1) * hg],
            )

    # software pipeline: PE stream is [fin1(c-1) mm, fin2(c-2) mm, conv(c) mms]
    # so every PE instruction is dep-ready when reached (no in-order stalls)
    for c in range(n_ch):
        if c >= 1:
            emit_fin1(c - 1)
        if c >= 2:
            emit_fin2(c - 2)
        emit_conv(c)
        if c >= 3:
            emit_out(c - 3)
    emit_fin1(n_ch - 1)
    for c in range(max(0, n_ch - 2), n_ch):
        emit_fin2(c)
    for c in range(max(0, n_ch - 3), n_ch):
        emit_out(c)


def build_program(n_ch=CPC, enable_asserts=False):
    nc = bacc.Bacc(
        "TRN2",
        debug=False,
        enable_asserts=enable_asserts,
        target_bir_lowering=False,
        num_devices=NCORES,
    )
    x_d = nc.dram_tensor("x", (n_ch, HP, N, WP), FP32R, kind="ExternalInput")
    a_d = nc.dram_tensor("a", (HP, n_ch, 3, W), FP32R, kind="ExternalInput")
    gb_d = nc.dram_tensor("gb", (1, 2 * n_ch), FP32, kind="ExternalInput")
    o_d = nc.dram_tensor("o", (n_ch, H, N, W), FP32, kind="ExternalOutput")
    with tile.TileContext(nc) as tc:
        with ExitStack() as ctx:
            _emit(ctx, tc, nc, x_d, a_d, gb_d, o_d, n_ch)
    nc.compile()
    return nc


def make_core_inputs(inputs, w, gamma, beta, k, n_ch=CPC):
    """Host-side shard prep for core k: padded x slab, banded A matrices, gamma/beta."""
    ch = slice(k * n_ch, (k + 1) * n_ch)
    xk = np.zeros((n_ch, HP, N, WP), np.float32)
    xk[:, 1:1 + H, :, 1:1 + W] = np.asarray(inputs[:, ch]).transpose(1, 2, 0, 3)
    wk = np.asarray(w[ch]).astype(np.float32)          # (n_ch, 1, 3, 3)
    ak = np.zeros((n_ch, 3, HP, W), np.float32)
    m = np.arange(W)
    for di in range(3):
        # A[c, dj, m+di, m] = w[c, 0, di, dj]
        ak[:, :, m + di, m] = wk[:, 0, di, :][:, :, None]
    ak = np.ascontiguousarray(ak.transpose(2, 0, 1, 3))  # (HP, n_ch, 3, W)
    gbk = np.concatenate(
        [np.asarray(gamma[ch]), np.asarray(beta[ch])]
    ).astype(np.float32).reshape(1, 2 * n_ch)
    return {"x": xk, "a": ak, "gb": gbk}


_PROGRAM = None


def kernel(inputs, w, b, gamma, beta):
    global _PROGRAM
    if _PROGRAM is None:
        _PROGRAM = build_program()
    inputs = np.asarray(inputs, np.float32)
    in_maps = [make_core_inputs(inputs, w, gamma, beta, k) for k in range(NCORES)]
    res = bass_utils.run_bass_kernel_spmd(_PROGRAM, in_maps, list(range(NCORES)))
    out = np.empty((N, C, H, W), np.float32)
    for k in range(NCORES):
        # per-core output is (CPC, H, N, W)
        out[:, k * CPC:(k + 1) * CPC] = res.results[k]["o"].transpose(2, 0, 1, 3)
    return out



# revision 15
# speedup vs baseline: 1.3250x; 1.3250x over previous
"""Depthwise 3x3 conv + sync BatchNorm (train mode) + ReLU6 on 8 Trainium2 cores.

Sharding: channels (192) split 24-per-core; per-channel independent, no
cross-core traffic.

v2 design (vs v1 baseline at ~403us):
  - bf16 input slabs, band matrices, y_sb, z and HBM output (host converts);
    PSUM accumulation stays fp32. Halves DMA bytes and SBUF traffic.
  - Input DMAs alternate between the two HWDGE queues (sync + scalar) to
    double input DMA engine fanout.
  - PSUM as two 4-bank mega-tiles [112, 4, 512] per channel; 12 matmuls each.
  - Drains are coarse ([112,k,448] APs): ScalarE drains 6 banks, DVE drains 2,
    each with accum_out giving free per-partition sum(y).
  - sum(y^2) via one DVE tensor_tensor_reduce over bf16 y_sb (2x mode).
  - Partition collapse of the 4 partial sums via an all-ones [112,112] lhsT
    matmul into the tail words of a PSUM bank: the result lands REPLICATED
    on all 112 partitions, so the whole scalar chain runs on [112,1] tiles
    and no separate broadcast matmul/copy is needed.
  - Output: ScalarE Relu(scale*y+bias) -> bf16 z, DVE min(z,6) (2x), SWDGE out.
"""

import numpy as np
import ml_dtypes
from contextlib import ExitStack

import concourse.bass as bass
import concourse.mybir as mybir
import concourse.tile as tile
from concourse import bacc, bass_utils
from concourse.bass_isa import ReduceOp

FP32 = mybir.dt.float32
BF16 = mybir.dt.bfloat16
AF = mybir.ActivationFunctionType
ALU = mybir.AluOpType

N, C, H, W = 32, 192, 112, 112
NCORES = 8
CPC = C // NCORES          # 24 channels per core
HP, WP = H + 2, W + 2      # zero-padded spatial dims
IPG = 4                    # images per PSUM group (448 fp32 <= 512 bank)
NF = IPG * W               # 448 matmul free dim
NTOT = N * H * W           # BN reduction size per channel (401408)
BN_EPS = 1e-5
BANK = 512                 # fp32 words per PSUM bank


def _emit(ctx: ExitStack, tc, nc, x_d, a_d, gb_d, o_d, n_ch):
    a_pool = ctx.enter_context(tc.tile_pool(name="a", bufs=1))
    const_pool = ctx.enter_context(tc.tile_pool(name="const", bufs=1))
    x_pool = ctx.enter_context(tc.tile_pool(name="x", bufs=3))
    y_pool = ctx.enter_context(tc.tile_pool(name="y", bufs=3))
    z_pool = ctx.enter_context(tc.tile_pool(name="z", bufs=2))
    st_pool = ctx.enter_context(tc.tile_pool(name="st", bufs=3))
    sc_pool = ctx.enter_context(tc.tile_pool(name="sc", bufs=3))
    psum_pool = ctx.enter_context(tc.tile_pool(name="pp", bufs=2, space="PSUM"))

    a_all = a_pool.tile([HP, n_ch, 3, W], BF16)
    nc.sync.dma_start(a_all[:], a_d.ap())
    gb = const_pool.tile([H, 2 * n_ch], FP32)
    nc.sync.dma_start(gb[:], gb_d.ap())
    eps_t = const_pool.tile([H, 1], FP32)      # BN eps as Sqrt bias operand
    nc.vector.memset(eps_t[:], BN_EPS)

    st = {c: {} for c in range(n_ch)}

    def emit_xdma(c):
        x_t = x_pool.tile([HP, N, WP], BF16)
        eng = nc.sync
        eng.dma_start(x_t[:], x_d.ap()[c])
        st[c]["x"] = x_t

    def emit_conv_half(c, half):
        # 4 groups x 3 taps into one 4-bank PSUM mega-tile
        x_t = st[c]["x"]
        mt = psum_pool.tile([H, 4, BANK], FP32, tag="mt")
        for g4 in range(4):
            g = half * 4 + g4
            for dj in range(3):
                nc.tensor.matmul(
                    mt[:, g4, 0:NF],
                    a_all[:, c, dj, :],
                    x_t[:, g * IPG:(g + 1) * IPG, dj:dj + W],
                    start=(dj == 0),
                    stop=(dj == 2),
                )
        st[c]["mA" if half == 0 else "mB"] = mt
        if half == 0:
            st[c]["y"] = y_pool.tile([H, 8, NF], BF16, tag="ysb", name="ysb")
            st[c]["bst"] = st_pool.tile([H, 8, 6], FP32, tag="bst", name="bst")

    def emit_drainA(c):
        # ScalarE: drain the 4 banks of mega A (one instr per bank)
        y_sb = st[c]["y"]
        for g in range(4):
            nc.scalar.activation(y_sb[:, g, :], st[c]["mA"][:, g, 0:NF], AF.Copy)

    def emit_statsA(c):
        for g in range(4):
            nc.vector.bn_stats(st[c]["bst"][:, g, :], st[c]["y"][:, g, :])

    def emit_drainB(c):
        y_sb, mB = st[c]["y"], st[c]["mB"]
        for g in range(4):
            nc.scalar.activation(y_sb[:, 4 + g, :], mB[:, g, 0:NF], AF.Copy)

    def emit_statsB(c):
        for g in range(4, 8):
            nc.vector.bn_stats(st[c]["bst"][:, g, :], st[c]["y"][:, g, :])

    def emit_aggr(c):
        # per-partition (mean, var) over the channel, then [mean, var, mean^2]
        stats3 = st_pool.tile([H, 3], FP32, tag="stats3", name="stats3")
        nc.vector.bn_aggr(stats3[:, 0:2], st[c]["bst"][:])
        nc.vector.tensor_scalar(
            stats3[:, 2:3], stats3[:, 0:1], stats3[:, 0:1], None, op0=ALU.mult
        )
        st[c]["stats3"] = stats3

    def emit_collapse(c):
        # cross-partition all-reduce on gpsimd: pst[p, j] = sum_k stats3[k, j]
        pst = sc_pool.tile([H, 3], FP32, tag="pst", name="pst")
        nc.gpsimd.partition_all_reduce(pst[:], st[c]["stats3"][:], H, ReduceOp.add)
        st[c]["pst"] = pst

    def emit_chain1(c):
        # replicated chain: em = [E[m], E[v], E[m^2]]; nvar = E[m]^2-E[v]-E[m^2]
        pst = st[c]["pst"]
        em = sc_pool.tile([H, 3], FP32, tag="em")
        nc.vector.tensor_scalar_mul(em[:], pst[:], 1.0 / H)
        nvar = sc_pool.tile([H, 1], FP32, tag="nvar")
        nc.vector.tensor_scalar(
            nvar[:], em[:, 0:1], em[:, 0:1], em[:, 1:2],
            op0=ALU.mult, op1=ALU.subtract,
        )
        nc.vector.tensor_scalar(
            nvar[:], nvar[:], em[:, 2:3], None, op0=ALU.subtract,
        )
        st[c].update(em=em, nvar=nvar)

    def emit_sqrt(c):
        std = sc_pool.tile([H, 1], FP32, tag="std")
        # std = Sqrt(-1 * (-var) + eps)
        nc.scalar.activation(
            std[:], st[c]["nvar"][:], AF.Sqrt, bias=eps_t[:, 0:1], scale=-1.0
        )
        st[c]["std"] = std

    def emit_chain2(c):
        em = st[c]["em"]
        istd = sc_pool.tile([H, 1], FP32, tag="istd")
        nc.vector.reciprocal(istd[:], st[c]["std"][:])
        scpair = sc_pool.tile([H, 2], FP32, tag="scpair")
        nc.vector.tensor_scalar(
            scpair[:, 0:1], istd[:], gb[:, c:c + 1], None, op0=ALU.mult
        )
        msc = sc_pool.tile([H, 1], FP32, tag="msc")
        nc.vector.tensor_scalar(
            msc[:], em[:, 0:1], scpair[:, 0:1], None, op0=ALU.mult
        )
        nc.vector.tensor_scalar(
            scpair[:, 1:2], gb[:, n_ch + c:n_ch + c + 1], msc[:], None,
            op0=ALU.subtract,
        )
        st[c]["scpair"] = scpair

    def emit_out(c, half):
        if half == 0:
            st[c]["z"] = z_pool.tile([H, 8, NF], BF16, tag="z", name="z")
        y_sb, z_sb, scpair = st[c]["y"], st[c]["z"], st[c]["scpair"]
        sl = slice(4 * half, 4 * half + 4)
        nc.scalar.activation(
            z_sb[:, sl, :].rearrange("p g f -> p (g f)"),
            y_sb[:, sl, :].rearrange("p g f -> p (g f)"), AF.Relu,
            bias=scpair[:, 1:2], scale=scpair[:, 0:1],
        )

    def emit_min_dma(c, half):
        z_sb = st[c]["z"]
        sl = slice(4 * half, 4 * half + 4)
        zf = z_sb[:, sl, :].rearrange("p g f -> p (g f)")
        nc.vector.tensor_scalar_min(zf, zf, 6.0)
        # SWDGE ring: keeps the HWDGE rings free for x prefetches
        nc.gpsimd.dma_start(
            o_d.ap()[c][:, 16 * half:16 * half + 16, :], z_sb[:, sl, :]
        )

    # software pipeline over channels
    emit_xdma(0)
    if n_ch > 1:
        emit_xdma(1)
    for i in range(n_ch + 2):
        c, p = i, i - 1          # conv channel, finalize channel
        if c < n_ch:
            if c + 2 < n_ch:
                emit_xdma(c + 2)
            if p >= 0:
                emit_collapse(p)
                emit_chain1(p)
                emit_sqrt(p)
                emit_chain2(p)
            emit_conv_half(c, 0)
            emit_drainA(c)
            emit_statsA(c)
            if p >= 0:
                emit_out(p, 0)
                emit_out(p, 1)
            emit_conv_half(c, 1)
            emit_drainB(c)
            emit_statsB(c)
            emit_aggr(c)
            if p >= 0:
                emit_min_dma(p, 0)
                emit_min_dma(p, 1)
        elif c == n_ch:
            # flush channel n_ch-1 fully
            emit_collapse(p)
            emit_chain1(p)
            emit_sqrt(p)
            emit_chain2(p)
            emit_out(p, 0)
            emit_out(p, 1)
            emit_min_dma(p, 0)
            emit_min_dma(p, 1)


def build_program(n_ch=CPC, enable_asserts=False):
    nc = bacc.Bacc(
        "TRN2",
        debug=False,
        enable_asserts=enable_asserts,
        target_bir_lowering=False,
        num_devices=NCORES,
    )
    x_d = nc.dram_tensor("x", (n_ch, HP, N, WP), BF16, kind="ExternalInput")
    a_d = nc.dram_tensor("a", (HP, n_ch, 3, W), BF16, kind="ExternalInput")
    gb_d = nc.dram_tensor("gb", (H, 2 * n_ch), FP32, kind="ExternalInput")
    o_d = nc.dram_tensor("o", (n_ch, H, N, W), BF16, kind="ExternalOutput")
    with tile.TileContext(nc) as tc:
        with ExitStack() as ctx:
            _emit(ctx, tc, nc, x_d, a_d, gb_d, o_d, n_ch)
    nc.compile()
    return nc


def make_core_inputs(inputs, w, gamma, beta, k, n_ch=CPC):
    """Host-side shard prep for core k: padded bf16 x slab, banded A, gamma/beta."""
    ch = slice(k * n_ch, (k + 1) * n_ch)
    xk = np.zeros((n_ch, HP, N, WP), ml_dtypes.bfloat16)
    xk[:, 1:1 + H, :, 1:1 + W] = (
        np.asarray(inputs[:, ch]).transpose(1, 2, 0, 3).astype(ml_dtypes.bfloat16)
    )
    wk = np.asarray(w[ch]).astype(np.float32)          # (n_ch, 1, 3, 3)
    ak = np.zeros((n_ch, 3, HP, W), np.float32)
    m = np.arange(W)
    for di in range(3):
        # A[c, dj, m+di, m] = w[c, 0, di, dj]
        ak[:, :, m + di, m] = wk[:, 0, di, :][:, :, None]
    ak = np.ascontiguousarray(ak.transpose(2, 0, 1, 3)).astype(ml_dtypes.bfloat16)
    gbk = np.concatenate(
        [np.asarray(gamma[ch]), np.asarray(beta[ch])]
    ).astype(np.float32).reshape(1, 2 * n_ch)
    gbk = np.ascontiguousarray(np.broadcast_to(gbk, (H, 2 * n_ch)))
    return {"x": xk, "a": ak, "gb": gbk}


_PROGRAM = None


def kernel(inputs, w, b, gamma, beta):
    global _PROGRAM
    if _PROGRAM is None:
        _PROGRAM = build_program()
    inputs = np.asarray(inputs, np.float32)
    in_maps = [make_core_inputs(inputs, w, gamma, beta, k) for k in range(NCORES)]
    res = bass_utils.run_bass_kernel_spmd(_PROGRAM, in_maps, list(range(NCORES)))
    out = np.empty((N, C, H, W), np.float32)
    for k in range(NCORES):
        # per-core output is (CPC, H, N, W) bf16
        ok = np.asarray(res.results[k]["o"]).astype(np.float32)
        out[:, k * CPC:(k + 1) * CPC] = ok.transpose(2, 0, 1, 3)
    return out
